# revision 12
# baseline (speedup 1.0000x reference)
"""MHA kernel for trn2: 8-core SPMD, core c = (batch c//2, head-group c%2 of 8 heads).

Per-core pipeline (all shapes hardcoded for B=4, S=2048, HIDDEN=1024, H=16, DK=DV=64):
  Phase 1: Q^T/K^T per head-pair [128, S] (bf16) and ones-augmented V
           [sk, 8, 128] (64 v-cols + 64 ones-cols) via PE (bf16 inputs and
           weights). Q/K biases fold into the ACT Identity-evacuation as
           per-partition bias vectors; V bias folds into the DVE evacuation as
           a host-replicated [128, 512] add. x/w are loaded as per-k tiles so
           the first matmul only waits on one [128, 1024] DMA; the big mask /
           wo DMAs are issued mid-phase-1.
  Phase 2: per sq-block j (512) per head: scores^T = K Q^T, exp on ACT
           (scale=1/8), mask multiply on DVE (bf16 mask), PV accumulation with
           the ones-augmented V so rows 64..127 of the PV psum hold the softmax
           denominator replicated across 64 partitions (matmul cost depends
           only on the moving dim, so the replication is free). PV runs TWO
           score tiles behind (lag-2) so the exp->mask latency never stalls
           the PE. At each head's end: stage the replicated denominator to
           SBUF (the custom-DVE reciprocal reads garbage from PSUM on HW),
           reciprocal_approx_fast, then one partition-aligned DVE multiply
           normalizes straight out of PSUM into oN.
  Phase 3: out-projection with pair-packed lhsT (K=128), drip-fed into the
           next j block as 8 half-steps between score tiles; the final block's
           steps run from the by-then-idle score PSUM pool so they pipeline.
  Host sums the 2 group partials per batch + bo.
"""

import numpy as np
import ml_dtypes

import concourse.bacc as bacc
import concourse.mybir as mybir
import concourse.tile as tile
from concourse.bass_utils import run_bass_kernel_spmd

B, S, HID, H = 4, 2048, 1024, 16
DK = DV = 64
G = 2              # head groups per batch (8 heads each)
HPC, PAIRS = 8, 4  # heads / head-pairs per core
SQB = 512          # sq block
NJ = S // SQB      # 4
NT = S // 128      # 16 sk tiles
KTN = HID // 128   # 8 hidden k-tiles
XHALF = S // 2

F32 = mybir.dt.float32
F32R = mybir.dt.float32r
BF16 = mybir.dt.bfloat16
AF = mybir.ActivationFunctionType


_NC = None


def _build_nc():
    nc = bacc.Bacc("TRN2")
    xq_d = nc.declare_dram_parameter("xqT", [HID, S], BF16, isOutput=False)
    xk_d = nc.declare_dram_parameter("xkT", [HID, S], BF16, isOutput=False)
    xv_d = nc.declare_dram_parameter("xvT", [HID, S], BF16, isOutput=False)
    mk_d = nc.declare_dram_parameter("maskJ", [NJ, S, SQB], BF16, isOutput=False)
    wq_d = nc.declare_dram_parameter("wq", [HID, 512], BF16, isOutput=False)
    wk_d = nc.declare_dram_parameter("wk", [HID, 512], BF16, isOutput=False)
    wv_d = nc.declare_dram_parameter("wv", [HID, 512], BF16, isOutput=False)
    bq_d = nc.declare_dram_parameter("bq", [128, PAIRS], F32, isOutput=False)
    bk_d = nc.declare_dram_parameter("bk", [128, PAIRS], F32, isOutput=False)
    bv_d = nc.declare_dram_parameter("bv", [128, 512], F32, isOutput=False)
    wo_d = nc.declare_dram_parameter("wo", [PAIRS, 128, HID], BF16, isOutput=False)
    out_d = nc.declare_dram_parameter("out", [S, HID], F32, isOutput=True)

    with tile.TileContext(nc) as tc:
        with tc.tile_pool(name="persist", bufs=1) as PP, \
             tc.tile_pool(name="wop", bufs=1) as WOP, \
             tc.tile_pool(name="mskp", bufs=2) as MP, \
             tc.tile_pool(name="ptp", bufs=5) as PTP, \
             tc.tile_pool(name="onp", bufs=2) as ONP, \
             tc.tile_pool(name="rcp", bufs=2) as RCP, \
             tc.tile_pool(name="obp", bufs=2) as OBP:
            qT = PP.tile([128, PAIRS, S], BF16, name="qT")
            kT = PP.tile([128, PAIRS, S], BF16, name="kT")
            vA = PP.tile([128, NT, HPC, 128], BF16, name="vA")
            bq_sb = PP.tile([128, PAIRS], F32, name="bq_sb")
            bk_sb = PP.tile([128, PAIRS], F32, name="bk_sb")
            bv_sb = PP.tile([128, 512], F32, name="bv_sb")
            # ones block: PV rows 64..127 become the denominator replicated
            nc.vector.memset(vA[:, :, :, DV:], 1.0)
            wo_sb = WOP.tile([128, PAIRS, HID], BF16, name="wo_sb")
            msk0 = MP.tile([128, NT, SQB], BF16, name="msk")

            # ---------------- Phase 1: projections ----------------
            with tc.tile_pool(name="xpool", bufs=16) as XP, \
                 tc.tile_pool(name="wpool", bufs=16) as WP, \
                 tc.tile_pool(name="ph1ps", bufs=2, space="PSUM") as PR:
                # Q and K: out[pair-dk 128, sq] += w.T @ x, bias on the evac
                first = True
                for xd, wd, brow, dstT in ((xq_d, wq_d, bq_sb, qT),
                                           (xk_d, wk_d, bk_sb, kT)):
                    w_sb = [WP.tile([128, 512], BF16, name="w_sb")
                            for _ in range(KTN)]
                    for k in range(KTN):
                        nc.sync.dma_start(w_sb[k][:], wd[k * 128:(k + 1) * 128, :])
                    for half in range(2):
                        x_sb = [XP.tile([128, XHALF], BF16, name="x_sb")
                                for _ in range(KTN)]
                        for k in range(KTN):
                            nc.sync.dma_start(
                                x_sb[k][:],
                                xd[k * 128:(k + 1) * 128,
                                   half * XHALF:(half + 1) * XHALF])
                        if first:
                            # small bias loads + the big mask DMA go after the
                            # first x/w descriptors so the PE starts early
                            nc.sync.dma_start(bq_sb[:], bq_d[:])
                            nc.sync.dma_start(bk_sb[:], bk_d[:])
                            nc.sync.dma_start(bv_sb[:], bv_d[:])
                            first = False
                        for nn in range(NJ // 2):
                            n = half * (NJ // 2) + nn
                            for hp in range(PAIRS):
                                ps = PR.tile([128, SQB], F32, name="ps_qk")
                                for k in range(KTN):
                                    nc.tensor.matmul(
                                        ps[:], w_sb[k][:, hp * 128:(hp + 1) * 128],
                                        x_sb[k][:, nn * SQB:(nn + 1) * SQB],
                                        start=(k == 0), stop=(k == KTN - 1))
                                nc.scalar.activation(
                                    dstT[:, hp, n * SQB:(n + 1) * SQB], ps[:],
                                    AF.Identity, bias=brow[:, hp:hp + 1])
                    if dstT is qT:
                        nc.sync.dma_start(
                            msk0[:], mk_d[0].rearrange("(t p) s -> p t s", p=128))

                # V: out[sk 128, head, dv] += x.T @ wv, bias on the DVE evac
                wv_sb = [WP.tile([128, 512], BF16, name="w_sb")
                         for _ in range(KTN)]
                for k in range(KTN):
                    nc.sync.dma_start(wv_sb[k][:], wv_d[k * 128:(k + 1) * 128, :])
                for hp in range(PAIRS):
                    nc.sync.dma_start(wo_sb[:, hp, :], wo_d[hp])
                for half in range(2):
                    xv_sb = [XP.tile([128, XHALF], BF16, name="x_sb")
                             for _ in range(KTN)]
                    for k in range(KTN):
                        nc.sync.dma_start(
                            xv_sb[k][:],
                            xv_d[k * 128:(k + 1) * 128,
                                 half * XHALF:(half + 1) * XHALF])
                    for stl in range(NT // 2):
                        st = half * (NT // 2) + stl
                        ps = PR.tile([128, HPC, DV], F32, name="ps_v")
                        for k in range(KTN):
                            nc.tensor.matmul(
                                ps[:], xv_sb[k][:, stl * 128:(stl + 1) * 128],
                                wv_sb[k][:], start=(k == 0), stop=(k == KTN - 1))
                        nc.vector.tensor_add(vA[:, st, :, 0:DV], ps[:], bv_sb[:])

            # ---------------- Phase 2+3: attention + out-projection ----------------
            # opps first: it inherits phase-1's PSUM banks but is first used a
            # full j-block later, so the first scores/PV matmuls start clean.
            with tc.tile_pool(name="opps", bufs=2, space="PSUM") as OPP, \
                 tc.tile_pool(name="pvps", bufs=2, space="PSUM") as PVP, \
                 tc.tile_pool(name="scps", bufs=2, space="PSUM") as SCP:

                def op_steps(j, oN, pool):
                    steps = []
                    for stl in range(4):
                        for nn in range(2):
                            def s_op(stl=stl, nn=nn):
                                st = 4 * j + stl
                                op = pool.tile([128, SQB], F32, name="op")
                                for hp in range(PAIRS):
                                    nc.tensor.matmul(
                                        op[:],
                                        oN[:, hp, stl * 128:(stl + 1) * 128],
                                        wo_sb[:, hp, nn * SQB:(nn + 1) * SQB],
                                        start=(hp == 0), stop=(hp == PAIRS - 1))
                                ob = OBP.tile([128, SQB], F32, name="ob")
                                nc.vector.tensor_copy(ob[:], op[:])
                                nc.sync.dma_start(
                                    out_d[st * 128:(st + 1) * 128,
                                          nn * SQB:(nn + 1) * SQB], ob[:])
                            steps.append(s_op)
                    return steps

                def emit_norm(ev):
                    # normalize out of PSUM: rows 64..127 hold the denominator
                    # replicated across partitions. The custom-DVE reciprocal
                    # needs SBUF operands, so stage the denominator first.
                    cpv, cpb, chp, coN = ev
                    dcp = RCP.tile([DV, SQB], F32, name="dcp")
                    nc.vector.tensor_copy(dcp[:], cpv[DV:128, :])
                    rc = RCP.tile([DV, SQB], F32, name="rc")
                    nc.vector.reciprocal_approx_fast(rc[:], dcp[:])
                    nc.vector.tensor_mul(coN[cpb:cpb + DV, chp, :],
                                         cpv[0:DV, :], rc[:])

                def emit_pv(c):
                    # PV matmuls for a score tile two iterations back; the
                    # lag-2 hides the full exp->mask latency from the PE.
                    cpv, cpt, ctt, chl, cpb, chp, coN, fin = c
                    # pre-issue the first weight load: the standalone LDW only
                    # depends on vA (resident), so it runs during the previous
                    # matmul instead of serializing behind this group's wait
                    nc.tensor.ldweights(vA[:, 2 * ctt, chl, :],
                                        tile_position=(0, 0))
                    for u in range(2):
                        nc.tensor.matmul(
                            cpv[:], vA[:, 2 * ctt + u, chl, :], cpt[:, u, :],
                            start=(ctt == 0 and u == 0), stop=(fin and u == 1))
                    return (cpv, cpb, chp, coN) if fin else None

                pend = None
                pend_pv = []
                for j in range(NJ):
                    if j == 0:
                        msk = msk0
                    else:
                        msk = MP.tile([128, NT, SQB], BF16, name="msk")
                        nc.sync.dma_start(
                            msk[:], mk_d[j].rearrange("(t p) s -> p t s", p=128))
                    oN = ONP.tile([128, PAIRS, SQB], BF16, name="oN")
                    steps = []
                    for hl in range(HPC):
                        hp, r = divmod(hl, 2)
                        pb = 64 * r
                        pv = PVP.tile([128, SQB], F32, name="pv")
                        for tt in range(NT // 2):
                            sc = SCP.tile([128, 2, SQB], F32, name="sc")
                            nc.tensor.ldweights(
                                kT[pb:pb + DK, hp, 2 * tt * 128:(2 * tt + 1) * 128],
                                tile_position=(pb, 0))
                            for u in range(2):
                                t = 2 * tt + u
                                nc.tensor.matmul(
                                    sc[:, u, :],
                                    kT[pb:pb + DK, hp, t * 128:(t + 1) * 128],
                                    qT[pb:pb + DK, hp, j * SQB:(j + 1) * SQB],
                                    start=True, stop=True)
                            ev = None
                            if len(pend_pv) >= 2:
                                ev = emit_pv(pend_pv.pop(0))
                            pt = PTP.tile([128, 2, SQB], BF16, name="pt")
                            nc.scalar.activation(pt[:], sc[:], AF.Exp, scale=0.125)
                            nc.vector.tensor_mul(pt[:], pt[:],
                                                 msk[:, 2 * tt:2 * tt + 2, :])
                            if ev is not None:
                                emit_norm(ev)
                            pend_pv.append((pv, pt, tt, hl, pb, hp, oN,
                                            tt == NT // 2 - 1))
                            # drip-feed the previous block's out-projection
                            # between score tiles
                            if pend is not None:
                                gi = hl * (NT // 2) + tt
                                if gi == 4:
                                    steps = op_steps(pend[0], pend[1], OPP)
                                elif gi >= 6 and (gi - 6) % 7 == 0 and steps:
                                    steps.pop(0)()
                    while steps:
                        steps.pop(0)()
                    pend = (j, oN)
                # final block's tail, nothing left to overlap with
                while pend_pv:
                    ev = emit_pv(pend_pv.pop(0))
                    if ev is not None:
                        emit_norm(ev)
                for s in op_steps(pend[0], pend[1], OPP):
                    s()
    nc.finalize()
    return nc


def get_nc():
    global _NC
    if _NC is None:
        _NC = _build_nc()
    return _NC


def make_in_maps(q_hidden_inputs, k_hidden_inputs, v_hidden_inputs, mask,
                 wq, bq, wk, bk, wv, bv, wo, bo):
    f32 = np.float32
    bf16 = ml_dtypes.bfloat16
    in_maps = []
    per_batch = []
    for b in range(B):
        xqT = np.ascontiguousarray(q_hidden_inputs[b].T).astype(bf16)
        xkT = np.ascontiguousarray(k_hidden_inputs[b].T).astype(bf16)
        xvT = np.ascontiguousarray(v_hidden_inputs[b].T).astype(bf16)
        maskT = mask[b].T.astype(bf16)                        # [sk, sq]
        maskJ = np.ascontiguousarray(
            maskT.reshape(S, NJ, SQB).transpose(1, 0, 2))     # [j, sk, 512]
        per_batch.append((xqT, xkT, xvT, maskJ))
    for c in range(2 * B):
        b, g = divmod(c, 2)
        xqT, xkT, xvT, maskJ = per_batch[b]
        hs = slice(g * HPC, (g + 1) * HPC)
        in_maps.append({
            "xqT": xqT, "xkT": xkT, "xvT": xvT, "maskJ": maskJ,
            "wq": np.ascontiguousarray(
                wq[hs].transpose(1, 0, 2).reshape(HID, 512)).astype(bf16),
            "wk": np.ascontiguousarray(
                wk[hs].transpose(1, 0, 2).reshape(HID, 512)).astype(bf16),
            "wv": np.ascontiguousarray(
                wv[hs].transpose(1, 0, 2).reshape(HID, 512)).astype(bf16),
            "bq": np.ascontiguousarray(
                bq[hs].reshape(PAIRS, 128).T, dtype=f32),
            "bk": np.ascontiguousarray(
                bk[hs].reshape(PAIRS, 128).T, dtype=f32),
            "bv": np.ascontiguousarray(
                np.tile(bv[hs].reshape(1, 512), (128, 1)), dtype=f32),
            "wo": np.ascontiguousarray(
                wo[g * 512:(g + 1) * 512, :].reshape(PAIRS, 128, HID)
            ).astype(bf16),
        })
    return in_maps


def assemble(results, bo):
    out = np.empty((B, S, HID), dtype=np.float32)
    for b in range(B):
        out[b] = results[2 * b]["out"] + results[2 * b + 1]["out"] \
            + bo.astype(np.float32)[None, :]
    return out


def run(inputs, trace=False, **kw):
    nc = get_nc()
    in_maps = make_in_maps(**inputs)
    bkr = run_bass_kernel_spmd(nc, in_maps, list(range(2 * B)), trace=trace, **kw)
    return assemble(bkr.results, np.asarray(inputs["bo"])), bkr


def kernel(**inputs):
    out, _ = run(inputs, trace=False)
    return out


# revision 15
# speedup vs baseline: 1.1561x; 1.1561x over previous
"""MHA kernel for trn2: 8-core SPMD, core c = (batch c//2, head-group c%2 of 8 heads).

Per-core pipeline (all shapes hardcoded for B=4, S=2048, HIDDEN=1024, H=16, DK=DV=64):
  Phase 1: Q^T/K^T per head-pair [128, S] (bf16) and ones-augmented V
           [sk, 8, 128] (64 v-cols + 64 ones-cols) via PE (bf16 inputs and
           weights). Q/K biases fold into the ACT Identity-evacuation as
           per-partition bias vectors; V bias folds into the DVE evacuation as
           a host-replicated [128, 512] add. x/w are loaded as per-k tiles so
           the first matmul only waits on one [128, 1024] DMA; the big mask /
           wo DMAs are issued mid-phase-1.
  Phase 2: per sq-block j (512) per head: scores^T = K Q^T, exp on ACT
           (scale=1/8), mask multiply on DVE (bf16 mask), PV accumulation with
           the ones-augmented V so rows 64..127 of the PV psum hold the softmax
           denominator replicated across 64 partitions (matmul cost depends
           only on the moving dim, so the replication is free). PV runs TWO
           score tiles behind (lag-2) so the exp->mask latency never stalls
           the PE. At each head's end: stage the replicated denominator to
           SBUF (the custom-DVE reciprocal reads garbage from PSUM on HW),
           reciprocal_approx_fast, then one partition-aligned DVE multiply
           normalizes straight out of PSUM into oN.
  Phase 3: out-projection with pair-packed lhsT (K=128), drip-fed into the
           next j block as 8 half-steps between score tiles; the final block's
           steps run from the by-then-idle score PSUM pool so they pipeline.
  Host sums the 2 group partials per batch + bo.
"""

import numpy as np
import ml_dtypes

import concourse.bacc as bacc
import concourse.mybir as mybir
import concourse.tile as tile
from concourse.bass_utils import run_bass_kernel_spmd

B, S, HID, H = 4, 2048, 1024, 16
DK = DV = 64
G = 2              # head groups per batch (8 heads each)
HPC, PAIRS = 8, 4  # heads / head-pairs per core
SQB = 512          # sq block
NJ = S // SQB      # 4
NT = S // 128      # 16 sk tiles
KTN = HID // 128   # 8 hidden k-tiles
XHALF = S // 2

F32 = mybir.dt.float32
F32R = mybir.dt.float32r
BF16 = mybir.dt.bfloat16
AF = mybir.ActivationFunctionType


_NC = None


def _build_nc():
    nc = bacc.Bacc("TRN2")
    xq_d = nc.declare_dram_parameter("xqT", [HID, S], BF16, isOutput=False)
    xk_d = nc.declare_dram_parameter("xkT", [HID, S], BF16, isOutput=False)
    xv_d = nc.declare_dram_parameter("xvT", [HID, S], BF16, isOutput=False)
    mk_d = nc.declare_dram_parameter("maskJ", [NJ, S, SQB], BF16, isOutput=False)
    wq_d = nc.declare_dram_parameter("wq", [HID, 512], BF16, isOutput=False)
    wk_d = nc.declare_dram_parameter("wk", [HID, 512], BF16, isOutput=False)
    wv_d = nc.declare_dram_parameter("wv", [HID, 512], BF16, isOutput=False)
    bq_d = nc.declare_dram_parameter("bq", [128, PAIRS], F32, isOutput=False)
    bk_d = nc.declare_dram_parameter("bk", [128, PAIRS], F32, isOutput=False)
    bv_d = nc.declare_dram_parameter("bv", [128, 512], F32, isOutput=False)
    wo_d = nc.declare_dram_parameter("wo", [PAIRS, 128, HID], BF16, isOutput=False)
    out_d = nc.declare_dram_parameter("out", [S, HID], F32, isOutput=True)

    with tile.TileContext(nc) as tc:
        with tc.tile_pool(name="persist", bufs=1) as PP, \
             tc.tile_pool(name="wop", bufs=1) as WOP, \
             tc.tile_pool(name="mskp", bufs=2) as MP, \
             tc.tile_pool(name="ptp", bufs=5) as PTP, \
             tc.tile_pool(name="onp", bufs=2) as ONP, \
             tc.tile_pool(name="rcp", bufs=2) as RCP, \
             tc.tile_pool(name="obp", bufs=2) as OBP:
            qT = PP.tile([128, PAIRS, S], BF16, name="qT")
            kT = PP.tile([128, PAIRS, S], BF16, name="kT")
            vA = PP.tile([128, NT, HPC, 128], BF16, name="vA")
            bq_sb = PP.tile([128, PAIRS], F32, name="bq_sb")
            bk_sb = PP.tile([128, PAIRS], F32, name="bk_sb")
            bv_sb = PP.tile([128, 512], F32, name="bv_sb")
            # ones block: PV rows 64..127 become the denominator replicated
            nc.vector.memset(vA[:, :, :, DV:], 1.0)
            wo_sb = WOP.tile([128, PAIRS, HID], BF16, name="wo_sb")
            msk0 = MP.tile([128, NT, SQB], BF16, name="msk")

            # ---------------- Phase 1: projections ----------------
            with tc.tile_pool(name="xpool", bufs=16) as XP, \
                 tc.tile_pool(name="wpool", bufs=16) as WP, \
                 tc.tile_pool(name="ph1ps", bufs=2, space="PSUM") as PR:
                # Q and K: out[pair-dk 128, sq] += w.T @ x, bias on the evac
                first = True
                for xd, wd, brow, dstT in ((xq_d, wq_d, bq_sb, qT),
                                           (xk_d, wk_d, bk_sb, kT)):
                    w_sb = [WP.tile([128, 512], BF16, name="w_sb")
                            for _ in range(KTN)]
                    for half in range(2):
                        x_sb = [XP.tile([128, XHALF], BF16, name="x_sb")
                                for _ in range(KTN)]
                        # interleave w[k]/x[k] issue so low-k tiles land first
                        # and the first matmul isn't stuck behind high-k DMAs
                        for k in range(KTN):
                            if half == 0:
                                nc.sync.dma_start(w_sb[k][:],
                                                  wd[k * 128:(k + 1) * 128, :])
                            nc.sync.dma_start(
                                x_sb[k][:],
                                xd[k * 128:(k + 1) * 128,
                                   half * XHALF:(half + 1) * XHALF])
                        if first:
                            # small bias loads + the big mask DMA go after the
                            # first x/w descriptors so the PE starts early
                            nc.sync.dma_start(bq_sb[:], bq_d[:])
                            nc.sync.dma_start(bk_sb[:], bk_d[:])
                            nc.sync.dma_start(bv_sb[:], bv_d[:])
                            first = False
                        for nn in range(NJ // 2):
                            n = half * (NJ // 2) + nn
                            for hp in range(PAIRS):
                                ps = PR.tile([128, SQB], F32, name="ps_qk")
                                for k in range(KTN):
                                    nc.tensor.matmul(
                                        ps[:], w_sb[k][:, hp * 128:(hp + 1) * 128],
                                        x_sb[k][:, nn * SQB:(nn + 1) * SQB],
                                        start=(k == 0), stop=(k == KTN - 1))
                                nc.scalar.activation(
                                    dstT[:, hp, n * SQB:(n + 1) * SQB], ps[:],
                                    AF.Identity, bias=brow[:, hp:hp + 1])
                    if dstT is qT:
                        nc.sync.dma_start(
                            msk0[:], mk_d[0].rearrange("(t p) s -> p t s", p=128))

                # V: out[sk 128, head, dv] += x.T @ wv, bias on the DVE evac
                wv_sb = [WP.tile([128, 512], BF16, name="w_sb")
                         for _ in range(KTN)]
                for k in range(KTN):
                    nc.sync.dma_start(wv_sb[k][:], wv_d[k * 128:(k + 1) * 128, :])
                for hp in range(PAIRS):
                    nc.sync.dma_start(wo_sb[:, hp, :], wo_d[hp])
                for half in range(2):
                    xv_sb = [XP.tile([128, XHALF], BF16, name="x_sb")
                             for _ in range(KTN)]
                    for k in range(KTN):
                        nc.sync.dma_start(
                            xv_sb[k][:],
                            xv_d[k * 128:(k + 1) * 128,
                                 half * XHALF:(half + 1) * XHALF])
                    for stl in range(NT // 2):
                        st = half * (NT // 2) + stl
                        ps = PR.tile([128, HPC, DV], F32, name="ps_v")
                        for k in range(KTN):
                            nc.tensor.matmul(
                                ps[:], xv_sb[k][:, stl * 128:(stl + 1) * 128],
                                wv_sb[k][:], start=(k == 0), stop=(k == KTN - 1))
                        nc.vector.tensor_add(vA[:, st, :, 0:DV], ps[:], bv_sb[:])

            # ---------------- Phase 2+3: attention + out-projection ----------------
            # opps first: it inherits phase-1's PSUM banks but is first used a
            # full j-block later, so the first scores/PV matmuls start clean.
            with tc.tile_pool(name="opps", bufs=2, space="PSUM") as OPP, \
                 tc.tile_pool(name="pvps", bufs=2, space="PSUM") as PVP, \
                 tc.tile_pool(name="scps", bufs=2, space="PSUM") as SCP:

                def op_steps(j, oN, pool):
                    steps = []
                    for stl in range(4):
                        for nn in range(2):
                            def s_op(stl=stl, nn=nn):
                                st = 4 * j + stl
                                op = pool.tile([128, SQB], F32, name="op")
                                for hp in range(PAIRS):
                                    nc.tensor.matmul(
                                        op[:],
                                        oN[:, hp, stl * 128:(stl + 1) * 128],
                                        wo_sb[:, hp, nn * SQB:(nn + 1) * SQB],
                                        start=(hp == 0), stop=(hp == PAIRS - 1))
                                ob = OBP.tile([128, SQB], F32, name="ob")
                                nc.vector.tensor_copy(ob[:], op[:])
                                nc.sync.dma_start(
                                    out_d[st * 128:(st + 1) * 128,
                                          nn * SQB:(nn + 1) * SQB], ob[:])
                            steps.append(s_op)
                    return steps

                def emit_norm(ev):
                    # normalize out of PSUM: rows 64..127 hold the denominator
                    # replicated across partitions. The custom-DVE reciprocal
                    # needs SBUF operands, so stage the denominator first.
                    cpv, cpb, chp, coN = ev
                    dcp = RCP.tile([DV, SQB], F32, name="dcp")
                    nc.vector.tensor_copy(dcp[:], cpv[DV:128, :])
                    rc = RCP.tile([DV, SQB], F32, name="rc")
                    nc.vector.reciprocal_approx_fast(rc[:], dcp[:])
                    nc.vector.tensor_mul(coN[cpb:cpb + DV, chp, :],
                                         cpv[0:DV, :], rc[:])

                def emit_pv(c):
                    # PV matmuls for a score tile two iterations back; the
                    # lag-2 hides the full exp->mask latency from the PE.
                    cpv, cpt, ctt, chl, cpb, chp, coN, fin = c
                    for u in range(2):
                        nc.tensor.matmul(
                            cpv[:], vA[:, 2 * ctt + u, chl, :], cpt[:, u, :],
                            start=(ctt == 0 and u == 0), stop=(fin and u == 1))
                    return (cpv, cpb, chp, coN) if fin else None

                pend = None
                pend_pv = []
                for j in range(NJ):
                    if j == 0:
                        msk = msk0
                    else:
                        msk = MP.tile([128, NT, SQB], BF16, name="msk")
                        nc.sync.dma_start(
                            msk[:], mk_d[j].rearrange("(t p) s -> p t s", p=128))
                    oN = ONP.tile([128, PAIRS, SQB], BF16, name="oN")
                    steps = []
                    for hl in range(HPC):
                        hp, r = divmod(hl, 2)
                        pb = 64 * r
                        pv = PVP.tile([128, SQB], F32, name="pv")
                        for tt in range(NT // 2):
                            sc = SCP.tile([128, 2, SQB], F32, name="sc")
                            for u in range(2):
                                t = 2 * tt + u
                                nc.tensor.matmul(
                                    sc[:, u, :],
                                    kT[pb:pb + DK, hp, t * 128:(t + 1) * 128],
                                    qT[pb:pb + DK, hp, j * SQB:(j + 1) * SQB],
                                    start=True, stop=True)
                            ev = None
                            if len(pend_pv) >= 2:
                                ev = emit_pv(pend_pv.pop(0))
                            pt = PTP.tile([128, 2, SQB], BF16, name="pt")
                            nc.scalar.activation(pt[:], sc[:], AF.Exp, scale=0.125)
                            nc.vector.tensor_mul(pt[:], pt[:],
                                                 msk[:, 2 * tt:2 * tt + 2, :])
                            if ev is not None:
                                emit_norm(ev)
                            pend_pv.append((pv, pt, tt, hl, pb, hp, oN,
                                            tt == NT // 2 - 1))
                            # drip-feed the previous block's out-projection
                            # between score tiles
                            if pend is not None:
                                gi = hl * (NT // 2) + tt
                                if gi == 4:
                                    steps = op_steps(pend[0], pend[1], OPP)
                                elif gi >= 6 and (gi - 6) % 7 == 0 and steps:
                                    steps.pop(0)()
                    while steps:
                        steps.pop(0)()
                    pend = (j, oN)
                # final block's tail, nothing left to overlap with
                while pend_pv:
                    ev = emit_pv(pend_pv.pop(0))
                    if ev is not None:
                        emit_norm(ev)
                for s in op_steps(pend[0], pend[1], OPP):
                    s()
    nc.finalize()
    return nc


def get_nc():
    global _NC
    if _NC is None:
        _NC = _build_nc()
    return _NC


def make_in_maps(q_hidden_inputs, k_hidden_inputs, v_hidden_inputs, mask,
                 wq, bq, wk, bk, wv, bv, wo, bo):
    f32 = np.float32
    bf16 = ml_dtypes.bfloat16
    in_maps = []
    per_batch = []
    for b in range(B):
        xqT = np.ascontiguousarray(q_hidden_inputs[b].T).astype(bf16)
        xkT = np.ascontiguousarray(k_hidden_inputs[b].T).astype(bf16)
        xvT = np.ascontiguousarray(v_hidden_inputs[b].T).astype(bf16)
        maskT = mask[b].T.astype(bf16)                        # [sk, sq]
        maskJ = np.ascontiguousarray(
            maskT.reshape(S, NJ, SQB).transpose(1, 0, 2))     # [j, sk, 512]
        per_batch.append((xqT, xkT, xvT, maskJ))
    for c in range(2 * B):
        b, g = divmod(c, 2)
        xqT, xkT, xvT, maskJ = per_batch[b]
        hs = slice(g * HPC, (g + 1) * HPC)
        in_maps.append({
            "xqT": xqT, "xkT": xkT, "xvT": xvT, "maskJ": maskJ,
            "wq": np.ascontiguousarray(
                wq[hs].transpose(1, 0, 2).reshape(HID, 512)).astype(bf16),
            "wk": np.ascontiguousarray(
                wk[hs].transpose(1, 0, 2).reshape(HID, 512)).astype(bf16),
            "wv": np.ascontiguousarray(
                wv[hs].transpose(1, 0, 2).reshape(HID, 512)).astype(bf16),
            "bq": np.ascontiguousarray(
                bq[hs].reshape(PAIRS, 128).T, dtype=f32),
            "bk": np.ascontiguousarray(
                bk[hs].reshape(PAIRS, 128).T, dtype=f32),
            "bv": np.ascontiguousarray(
                np.tile(bv[hs].reshape(1, 512), (128, 1)), dtype=f32),
            "wo": np.ascontiguousarray(
                wo[g * 512:(g + 1) * 512, :].reshape(PAIRS, 128, HID)
            ).astype(bf16),
        })
    return in_maps


def assemble(results, bo):
    out = np.empty((B, S, HID), dtype=np.float32)
    for b in range(B):
        out[b] = results[2 * b]["out"] + results[2 * b + 1]["out"] \
            + bo.astype(np.float32)[None, :]
    return out


def run(inputs, trace=False, **kw):
    nc = get_nc()
    in_maps = make_in_maps(**inputs)
    bkr = run_bass_kernel_spmd(nc, in_maps, list(range(2 * B)), trace=trace, **kw)
    return assemble(bkr.results, np.asarray(inputs["bo"])), bkr


def kernel(**inputs):
    out, _ = run(inputs, trace=False)
    return out
